# revision 1
# baseline (speedup 1.0000x reference)
"""Trainium2 Bass kernel v2 for nn_Middle_Moudle_v3 (retrieval_knn).

Per episode (b, s): cosine similarity of every support spatial C-vector
against every query spatial C-vector, max over query positions.

  support_x, query_x: [8, 75, 64, 19, 19] fp32  ->  out [8, 75, 361] fp32

Data-parallel over batch (8 episodes -> 8 cores). Host packs inputs as
bf16 [128, 38*362]: tile j holds pairs (2j, 2j+1) in partition halves
(channel c of pair 2j+e at partition 64e+c), query position n in columns
(n=361 duplicates n=360; pair 75 duplicates pair 74 - both max-harmless).

Per-core pipeline (3 tile groups of 13/13/12 for phase overlap):
  - ACT squares (bf16) -> PE shifted-window ones-matmul accumulates
    per-pair sumsq rows [2T, 362] in PSUM (K=128, also warms the HAM
    clock gate: K=64-only activity never reaches 2.4 GHz)
  - ACT Ln/Exp(-0.5): rq rows bf16, rs rows fp32 (shipped raw to host)
  - SBUF->SBUF broadcast DMA expands rq rows to qh layout [128, T*362]
  - DVE qh = q * rq2 (one bf16 2x multiply per group)
  - PE cosine GEMM per pair: 3 M-chunks, K=64, moving dim 362, bf16;
    batched by partition-quadrant (all e=0 pairs of a group, then e=1)
    to avoid the quadrant-source race seen with per-matmul alternation
  - reduce, split to balance DVE vs ACT:
      direct pairs:  DVE tensor_reduce max [128,3,362] PSUM -> colmax
      fold pairs:    ACT copy PSUM -> SBUF bf16, then DVE overlap-fold
                     max chain (2x bf16) + batched tail reduce
  - colmax [128, 228] + rs rows go to DRAM raw; host applies rs and
    reassembles (support-norm scale commutes with the max)
"""
import numpy as np
import ml_dtypes

import concourse.bass as bass
import concourse.mybir as mybir
import concourse.tile as tile
from concourse.bass_utils import run_bass_kernel_spmd

F32 = mybir.dt.float32
BF16 = mybir.dt.bfloat16
B = 8
S = 75
C = 64
N = 361
N2 = 362
SP = 76        # padded pairs
NT = 38        # two-pair tiles
NP = 75        # computed pairs (pad pair 75 skipped)
GROUPS = [(0, 16), (16, 16), (32, 6)]
CHUNKS = [(0, 128), (128, 128), (256, 105)]
# per (group_index, e) quadrant-run: how many leading pairs reduce directly
# on DVE (tensor_reduce from PSUM); the rest go through the ACT-copy +
# DVE-fold path.  Tuned to balance DVE vs ACT busy time.
HEAT = 20      # heater matmuls (row-tiled pairs) to warm the PE clock

_ws_ctr = [0]


def _split_multi_waits(nc):
    """Move all-but-one sync wait of each instruction onto injected
    InstEventSemaphore instructions (standalone sequencer waits)."""
    for f in nc.m.functions:
        for bb in f.blocks:
            insts = list(bb.instructions)
            out = []
            changed = False
            for ins in insts:
                si = ins.sync_info
                if si is not None and len(si.on_wait) > 1:
                    waits = list(si.on_wait)
                    for w in waits[:-1]:
                        _ws_ctr[0] += 1
                        ev = mybir.InstEventSemaphore(
                            name=f"wsplit_{_ws_ctr[0]}",
                            engine=ins.engine,
                            sync_info=mybir.SyncInfo(on_wait=[w], on_update=[]),
                        )
                        out.append(ev)
                    ins.sync_info = mybir.SyncInfo(
                        on_wait=[waits[-1]], on_update=list(si.on_update)
                    )
                    changed = True
                out.append(ins)
            if changed:
                bb.instructions = out


def _build_nc(repeats=None):
    win_np = np.zeros((128, 152), dtype=np.float32)
    win_np[0:64, 74] = 1.0
    win_np[64:128, 75] = 1.0

    nc = bass.Bass(target_bir_lowering=False)
    sup_d = nc.dram_tensor("support", [128, NT * N2], BF16, kind="ExternalInput")
    qry_d = nc.dram_tensor("query", [128, NT * N2], BF16, kind="ExternalInput")
    cmax_d = nc.dram_tensor("cmax", [128, 3 * SP], F32, kind="ExternalOutput")
    rs_d = nc.dram_tensor("rs", [SP, N2], F32, kind="ExternalOutput")
    rqs_d = nc.dram_tensor("rq_scr", [SP, N2], BF16)
    win_d = nc.inline_tensor(win_np, name="win")

    with tile.TileContext(nc) as tc:
        with tc.tile_pool(name="inp", bufs=1) as inp, \
             tc.tile_pool(name="work", bufs=1) as work, \
             tc.tile_pool(name="sqp", bufs=1) as sqp, \
             tc.tile_pool(name="rqp", bufs=1) as rqp, \
             tc.tile_pool(name="qhp", bufs=2) as qhp, \
             tc.tile_pool(name="fbp", bufs=2) as fbp, \
             tc.tile_pool(name="psq", bufs=1, space="PSUM") as psq, \
             tc.tile_pool(name="psd", bufs=2, space="PSUM") as psd:

            win32 = work.tile([128, 152], F32)
            nc.sync.dma_start(win32[:], win_d[:])
            win_sb = work.tile([128, 152], BF16)
            nc.vector.tensor_copy(win_sb[:], win32[:])

            st = inp.tile([128, NT, N2], BF16)
            qt = inp.tile([128, NT, N2], BF16)
            for (g0, T) in GROUPS:
                a, b = g0 * N2, (g0 + T) * N2
                nc.scalar.dma_start(st[:, g0:g0 + T, :], sup_d[:, a:b])
                nc.scalar.dma_start(qt[:, g0:g0 + T, :], qry_d[:, a:b])

            rq_rows = work.tile([SP, N2], BF16)
            rs_rows = work.tile([SP, N2], F32)
            lnt = work.tile([SP, N2], F32)
            colmax = work.tile([128, SP, 3], F32)

            def body():
                _body(nc, tc, st, qt, win_sb, rq_rows, rs_rows, lnt, colmax,
                      sqp, rqp, qhp, fbp, psq, psd, cmax_d, rs_d, rqs_d)

            if repeats is None:
                body()
            else:
                with tc.For_i(0, repeats, 1):
                    body()

    _split_multi_waits(nc)
    return nc


def _body(nc, tc, st, qt, win_sb, rq_rows, rs_rows, lnt, colmax,
          sqp, rqp, qhp, fbp, psq, psd, cmax_d, rs_d, rqs_d):
    # --- HAM heater: K=64-only streams never lift the PE clock gate out of
    # its 1.2 GHz idle state; a burst of concurrent row-tiled pairs (full
    # array activity) flips it to 2.4 GHz.  Results are garbage and land in
    # the psq bank slot, which the first sumsq matmul overwrites.
    hbank = psq.tile([128, 2, 512], F32, tag="bank", name="hbank")
    for h in range(HEAT):
        e = h % 2
        nc.tensor.matmul(hbank[:, e, 0:152],
                         win_sb[64 * e:64 * e + 64, 0:128],
                         win_sb[64 * e:64 * e + 64, 0:152],
                         start=True, stop=True)

    state = {}
    shared_pend = []

    def pre_squares(gi):
        g0, T = GROUPS[gi]
        sq_s = sqp.tile([128, 16, N2], BF16, tag="sq_s", name=f"sqs{gi}")
        sq_q = sqp.tile([128, 16, N2], BF16, tag="sq_q", name=f"sqq{gi}")
        nc.scalar.square(sq_s[:, 0:T, :], st[:, g0:g0 + T, :])
        nc.vector.tensor_tensor(out=sq_q[:, 0:T, :], in0=qt[:, g0:g0 + T, :],
                                in1=qt[:, g0:g0 + T, :],
                                op=mybir.AluOpType.mult)
        bank = psq.tile([128, 2, 512], F32, tag="bank", name=f"bank{gi}")
        state[gi] = (sq_s, sq_q, bank)

    def pre_sumsq_mm(gi, l):
        g0, T = GROUPS[gi]
        sq_s, sq_q, bank = state[gi]
        nr = 2 * T
        lhsT = win_sb[:, 74 - 2 * l:74 - 2 * l + nr]
        nc.tensor.matmul(bank[0:nr, 0, 0:N2], lhsT, sq_s[:, l, :],
                         start=(l == 0), stop=(l == T - 1))
        nc.tensor.matmul(bank[0:nr, 1, 0:N2], lhsT, sq_q[:, l, :],
                         start=(l == 0), stop=(l == T - 1))

    def pre_norms(gi):
        g0, T = GROUPS[gi]
        bank = state[gi][2]
        r0, nr = 2 * g0, 2 * T
        nc.scalar.activation(lnt[r0:r0 + nr, :], bank[0:nr, 1, 0:N2],
                             mybir.ActivationFunctionType.Ln)
        nc.scalar.activation(rq_rows[r0:r0 + nr, :], lnt[r0:r0 + nr, :],
                             mybir.ActivationFunctionType.Exp, scale=-0.5)
        nc.scalar.activation(lnt[r0:r0 + nr, :], bank[0:nr, 0, 0:N2],
                             mybir.ActivationFunctionType.Ln)
        nc.scalar.activation(rs_rows[r0:r0 + nr, :], lnt[r0:r0 + nr, :],
                             mybir.ActivationFunctionType.Exp, scale=-0.5)
        # rq rows: DRAM bounce + broadcast read into qh layout (same sync
        # queue: FIFO orders write before read)
        nc.sync.dma_start(rqs_d[r0:r0 + nr, :], rq_rows[r0:r0 + nr, :])
        rq2 = rqp.tile([128, 16, N2], BF16, tag="rq2", name=f"rq2{gi}")
        for e in range(2):
            src_rows = rqs_d[r0 + e:r0 + e + 1, :]
            src_ap = bass.AP(tensor=src_rows.tensor, offset=src_rows.offset,
                             ap=[[0, 64], [2 * N2, T], [1, N2]])
            nc.sync.dma_start(rq2[64 * e:64 * e + 64, 0:T, :], src_ap)
        state[gi] = state[gi][:2] + (bank, rq2)

    def gemm_reduce(gi, interleave=None):
        # Reduce-path assignment: group 2 (small, last) is all-direct so its
        # TRs pipeline with its own GEMMs (no fold tail after the last
        # matmul).  Groups 0/1: e=0 even-l pairs and the final e=1 pair are
        # direct (DVE); the rest fold (ACT copy + DVE fold chain).  The
        # first 8-pair fbB fold chain is spread one instruction per cycle
        # mid-phase instead of clumping at the group boundary.
        g0, T = GROUPS[gi]
        rq2 = state[gi][3]
        qh = qhp.tile([128, 16, N2], BF16, tag="qh", name=f"qh{gi}")
        h = (T + 1) // 2
        nc.vector.tensor_tensor(out=qh[:, 0:h, :], in0=qt[:, g0:g0 + h, :],
                                in1=rq2[:, 0:h, :], op=mybir.AluOpType.mult)
        nc.vector.tensor_tensor(out=qh[:, h:T, :],
                                in0=qt[:, g0 + h:g0 + T, :],
                                in1=rq2[:, h:T, :], op=mybir.AluOpType.mult)
        il = list(interleave) if interleave else []
        ic = 0
        heat = psq.tile([128, 2, 512], F32, tag="bank", name=f"heat{gi}") \
            if not il else None
        all_direct = (gi == 2)
        pend = shared_pend
        fbA = fbB = None
        if not all_direct:
            fbA = fbp.tile([128, 16, 3, N2], BF16, tag="fb", name=f"fbA{gi}")
            fbB = fbp.tile([128, 16, 3, N2], BF16, tag="fb", name=f"fbB{gi}")
        for l in range(T):
            for e in range(2):
                j = g0 + l
                P = 2 * j + e
                if P >= NP:
                    continue
                dot = psd.tile([128, 3, 512], F32, tag="dot")
                for m, (off, mc) in enumerate(CHUNKS):
                    nc.tensor.matmul(
                        dot[0:mc, m, 0:N2],
                        st[64 * e:64 * e + 64, j, off:off + mc],
                        qh[64 * e:64 * e + 64, l, 0:N2],
                        start=True, stop=True,
                    )
                direct = all_direct or (e == 0 and l % 2 == 0) or \
                    (e == 1 and l == 15)
                if direct:
                    nc.vector.tensor_reduce(
                        colmax[:, P, :], dot[:, :, 0:N2],
                        axis=mybir.AxisListType.X, op=mybir.AluOpType.max)
                elif e == 0:
                    nc.scalar.copy(fbA[:, l // 2, :, :], dot[:, :, 0:N2])
                else:
                    nc.scalar.copy(fbB[:, l, :, :], dot[:, :, 0:N2])
            if not all_direct and l == 8:
                pend += _fold_steps(nc, fbB[:, 0:8, :, :], colmax,
                                    2 * g0 + 1, 8, stride=6)
            if pend:
                pend.pop(0)()
                if gi == 2 and pend:
                    pend.pop(0)()
            # keep the PE fed + HAM warm: next group's K=128 sumsq, else
            # garbage row-tiled heater pairs (full-array activity)
            if ic < len(il):
                il[ic]()
                ic += 1
            elif heat is not None and l % 4 == 1:
                for he in range(2):
                    nc.tensor.matmul(
                        heat[:, he, 0:N2],
                        st[64 * he:64 * he + 64, (g0 + l) % NT, 0:128],
                        qt[64 * he:64 * he + 64, (g0 + l) % NT, 0:N2],
                        start=True, stop=True)
        if not all_direct:
            pend += _fold_steps(nc, fbA, colmax, 2 * g0 + 2, (T + 1) // 2,
                                stride=12)
            pend += _fold_steps(nc, fbB[:, 8:15, :, :], colmax,
                                2 * (g0 + 8) + 1, 7, stride=6)
        if gi == 2:
            for fn in pend:
                fn()
            del pend[:]
        while ic < len(il):
            il[ic]()
            ic += 1

    def runB_len(g0, T):
        return [l for l in range(T) if 2 * (g0 + l) + 1 < NP]

    # software pipeline: keep the PE queue stocked with independent work so
    # the norms->bounce->qh latency chain of each group hides behind other
    # groups' matmuls.
    pre_squares(0)
    for l in range(GROUPS[0][1]):
        pre_sumsq_mm(0, l)
    pre_norms(0)
    pre_squares(1)
    for l in range(GROUPS[1][1]):
        pre_sumsq_mm(1, l)
    pre_norms(1)

    gemm_reduce(0)
    pre_squares(2)
    il2 = [(lambda l=l: pre_sumsq_mm(2, l)) for l in range(GROUPS[2][1])]
    gemm_reduce(1, interleave=il2)
    pre_norms(2)
    # group 0's deferred folds all popped during group 1's cycles
    nc.scalar.dma_start(cmax_d[:, 0:96], colmax[:, 0:32, :])
    gemm_reduce(2)
    nc.scalar.dma_start(cmax_d[:, 96:228], colmax[:, 32:76, :])

    nc.scalar.dma_start(rs_d[:], rs_rows[:])


def _fold_steps(nc, fb, colmax, P0, nf, stride=6):
    """Like _fold but returns a list of single-instruction closures so the
    chain can be spread across pipeline cycles."""
    mx = mybir.AluOpType.max
    steps = []
    for (hi, w) in ((180, 182), (90, 92), (46, 46), (22, 24)):
        steps.append(lambda hi=hi, w=w: nc.vector.tensor_tensor(
            out=fb[:, 0:nf, :, 0:w], in0=fb[:, 0:nf, :, 0:w],
            in1=fb[:, 0:nf, :, hi:hi + w], op=mx))
    def tail():
        cv = colmax[:, P0, :]
        out_ap = bass.AP(tensor=cv.tensor, offset=cv.offset,
                         ap=[list(cv.ap[0]), [stride, nf], [1, 3]])
        nc.vector.tensor_reduce(out_ap, fb[:, 0:nf, :, 0:24],
                                axis=mybir.AxisListType.X, op=mx)
    steps.append(tail)
    return steps


def _fold(nc, fb, colmax, P0, nf, stride=6):
    """Overlap-fold max chain over query positions, then batched reduce.

    fb[:, k, m, :] holds the dot bank of the k-th batch pair, chunk m.
    Writes colmax[:, P0 + k*stride//3, m] via a strided view.
    """
    if nf <= 0:
        return
    mx = mybir.AluOpType.max
    for (hi, w) in ((180, 182), (90, 92), (46, 46), (22, 24)):
        nc.vector.tensor_tensor(out=fb[:, 0:nf, :, 0:w],
                                in0=fb[:, 0:nf, :, 0:w],
                                in1=fb[:, 0:nf, :, hi:hi + w], op=mx)
    cv = colmax[:, P0, :]
    out_ap = bass.AP(tensor=cv.tensor, offset=cv.offset,
                     ap=[list(cv.ap[0]), [stride, nf], [1, 3]])
    nc.vector.tensor_reduce(out_ap, fb[:, 0:nf, :, 0:24],
                            axis=mybir.AxisListType.X, op=mx)


_NC_CACHE = None


def _get_nc():
    global _NC_CACHE
    if _NC_CACHE is None:
        _NC_CACHE = _build_nc()
    return _NC_CACHE


def _pack(x):
    """[B, S, C, ...spatial] fp32 -> [B, 128, NT*N2] bf16 (pad dup)."""
    x = np.asarray(x, dtype=np.float32).reshape(B, S, C, N)
    x = np.concatenate([x, x[:, S - 1:S]], axis=1)          # pair 75 = dup 74
    x = np.concatenate([x, x[:, :, :, N - 1:N]], axis=3)    # col 361 = dup 360
    x = x.reshape(B, NT, 2, C, N2).transpose(0, 2, 3, 1, 4)  # [B,2,C,NT,N2]
    x = x.reshape(B, 128, NT * N2)
    return np.ascontiguousarray(x.astype(ml_dtypes.bfloat16))


def kernel(support_x, query_x, **_unused):
    sup = _pack(support_x)
    qry = _pack(query_x)

    nc = _get_nc()
    in_maps = [{"support": sup[b], "query": qry[b]} for b in range(B)]
    res = run_bass_kernel_spmd(nc, in_maps, core_ids=list(range(B)))

    i = np.arange(N)
    m, r = i // 128, i % 128
    out = np.empty((B, S, N), dtype=np.float32)
    for b in range(B):
        cm = np.asarray(res.results[b]["cmax"]).reshape(128, SP, 3)
        rs = np.asarray(res.results[b]["rs"])
        out[b] = cm[r, :, m].T[0:S] * rs[0:S, 0:N]
    return np.ascontiguousarray(out)



# revision 2
# speedup vs baseline: 1.6262x; 1.6262x over previous
"""Trainium2 Bass kernel v3 for nn_Middle_Moudle_v3 (retrieval_knn).

Per episode (b, s): cosine similarity of every support spatial C-vector
against every query spatial C-vector, max over query positions.

  support_x, query_x: [8, 75, 64, 19, 19] fp32  ->  out [8, 75, 361] fp32

Data-parallel over batch (8 episodes -> 8 cores).

v3 design notes (vs v2):
  - Query is normalized ON HOST (fp32) and shipped as q-hat bf16; support
    norms are applied on host after the kernel (scale commutes with max).
    The entire on-device norm pipeline (sumsq matmuls, squares, Ln/Exp,
    rq broadcast DMA, qh multiplies) disappears.
  - HAM fix: K=64 half-array matmuls never register as PE activity, so
    the clock gate sat at 1.2 GHz for the whole main phase in v2 (trace:
    K=4/8 for the last 135us, 362ns/MM vs 184ns warm).  v3 stores the
    support twice with the opposite partition half zeroed (sup0: pairs
    2j in partitions 0-63, zeros below; sup1: pairs 2j+1 in 64-127,
    zeros above).  Every main matmul is then a genuine K=128 full-array
    op (the zero half contributes 0 against the other pair's q-hat),
    which keeps HAM at 2.4 GHz.  Same DMA bytes; zeros via one-time
    memsets overlapped with the input DMA.
  - Drain split tuned from trace: P%3==0 pairs reduce directly on DVE
    from PSUM (1546ns); the other two of every three are ACT-copied to
    SBUF bf16 (1350ns) and max-folded on DVE (~590ns amortized), so the
    two drain engines run concurrently at similar load.
"""
import numpy as np
import ml_dtypes

import concourse.bass as bass
import concourse.mybir as mybir
import concourse.tile as tile
from concourse.bass_utils import run_bass_kernel_spmd

F32 = mybir.dt.float32
BF16 = mybir.dt.bfloat16
B = 8
S = 75
C = 64
N = 361
N2 = 362
SP = 76        # padded pairs
NT = 38        # two-pair tiles
NP = 75        # computed pairs (pad pair 75 skipped)
CHUNKS = [(0, 128), (128, 128), (256, 105)]
DGROUPS = [(0, 13), (13, 13), (26, 12)]
HEAT = 26      # K=128 heater matmuls to warm the PE clock during input DMA
EPS = 1e-8

# fold batches: 12 consecutive pairs -> P%3==0 direct (4), P%3 in {1,2}
# fold (8) into one fb tile; slots 0-3 = P%3==1, slots 4-7 = P%3==2.
FOLDB = 12

_ws_ctr = [0]


def _split_multi_waits(nc):
    """Move all-but-one sync wait of each instruction onto injected
    InstEventSemaphore instructions (standalone sequencer waits)."""
    for f in nc.m.functions:
        for bb in f.blocks:
            insts = list(bb.instructions)
            out = []
            changed = False
            for ins in insts:
                si = ins.sync_info
                if si is not None and len(si.on_wait) > 1:
                    waits = list(si.on_wait)
                    for w in waits[:-1]:
                        _ws_ctr[0] += 1
                        ev = mybir.InstEventSemaphore(
                            name=f"wsplit_{_ws_ctr[0]}",
                            engine=ins.engine,
                            sync_info=mybir.SyncInfo(on_wait=[w], on_update=[]),
                        )
                        out.append(ev)
                    ins.sync_info = mybir.SyncInfo(
                        on_wait=[waits[-1]], on_update=list(si.on_update)
                    )
                    changed = True
                out.append(ins)
            if changed:
                bb.instructions = out


def _build_nc():
    win_np = np.zeros((128, 152), dtype=np.float32)
    win_np[:, 74] = 1.0

    nc = bass.Bass(target_bir_lowering=False)
    sup_d = nc.dram_tensor("support", [128, NT * N2], BF16, kind="ExternalInput")
    qry_d = nc.dram_tensor("query", [128, NT * N2], BF16, kind="ExternalInput")
    cmax_d = nc.dram_tensor("cmax", [128, 3 * SP], F32, kind="ExternalOutput")
    win_d = nc.inline_tensor(win_np, name="win")

    mx = mybir.AluOpType.max

    with tile.TileContext(nc) as tc:
        with tc.tile_pool(name="inp", bufs=1) as inp, \
             tc.tile_pool(name="work", bufs=1) as work, \
             tc.tile_pool(name="fbp", bufs=2) as fbp, \
             tc.tile_pool(name="psh", bufs=1, space="PSUM") as psh, \
             tc.tile_pool(name="psd", bufs=2, space="PSUM") as psd:

            # --- heater weights (K=128) land first; heat the PE clock
            # while the big input DMAs stream in.
            win32 = work.tile([128, 152], F32)
            nc.sync.dma_start(win32[:], win_d[:])
            win_sb = work.tile([128, 152], BF16)
            nc.vector.tensor_copy(win_sb[:], win32[:])

            sup0 = inp.tile([128, NT, N2], BF16)
            sup1 = inp.tile([128, NT, N2], BF16)
            qt = inp.tile([128, NT, N2], BF16)

            # zero the dead partition halves once (overlaps input DMA)
            nc.vector.memset(sup0[64:128, :, :], 0.0)
            nc.gpsimd.memset(sup1[0:64, :, :], 0.0)

            for (g0, T) in DGROUPS:
                a, b = g0 * N2, (g0 + T) * N2
                nc.sync.dma_start(sup0[0:64, g0:g0 + T, :], sup_d[0:64, a:b])
                nc.sync.dma_start(sup1[64:128, g0:g0 + T, :],
                                  sup_d[64:128, a:b])
                nc.gpsimd.dma_start(qt[:, g0:g0 + T, :], qry_d[:, a:b])

            colmax = work.tile([128, SP, 3], F32)

            # --- heater: full-array K=128 matmuls into a scrap bank
            hbank = psh.tile([128, 512], F32)
            for h in range(HEAT):
                nc.tensor.matmul(hbank[:, 0:152], win_sb[:, 0:128],
                                 win_sb[:, 0:152], start=True, stop=True)

            pend = []

            def fold_batch(fb, bi):
                P0 = FOLDB * bi
                steps = []
                for (hi, w) in ((180, 182), (90, 92), (46, 46), (22, 24)):
                    steps.append(lambda hi=hi, w=w: nc.vector.tensor_tensor(
                        out=fb[:, :, :, 0:w], in0=fb[:, :, :, 0:w],
                        in1=fb[:, :, :, hi:hi + w], op=mx))

                def tail(t):
                    cv = colmax[:, P0 + 1 + t, :]
                    out_ap = bass.AP(tensor=cv.tensor, offset=cv.offset,
                                     ap=[list(cv.ap[0]), [9, 4], [1, 3]])
                    nc.vector.tensor_reduce(out_ap, fb[:, 4 * t:4 * t + 4, :, 0:24],
                                            axis=mybir.AxisListType.X, op=mx)
                steps.append(lambda: tail(0))
                steps.append(lambda: tail(1))
                return steps

            fb = None
            for j in range(NT):
                for e in range(2):
                    P = 2 * j + e
                    if P >= NP:
                        continue
                    supE = sup0 if e == 0 else sup1
                    dot = psd.tile([128, 3, 512], F32, tag="dot")
                    for m, (off, mc) in enumerate(CHUNKS):
                        nc.tensor.matmul(
                            dot[0:mc, m, 0:N2],
                            supE[:, j, off:off + mc],
                            qt[:, j, 0:N2],
                            start=True, stop=True,
                        )
                    r = P % 3
                    bi = P // FOLDB
                    if P % FOLDB == 0 and P + 1 < NP and bi < 6:
                        fb = fbp.tile([128, 8, 3, N2], BF16, tag="fb",
                                      name=f"fb{bi}")
                    if r == 0 or bi >= 6:
                        nc.vector.tensor_reduce(
                            colmax[:, P, :], dot[:, :, 0:N2],
                            axis=mybir.AxisListType.X, op=mx)
                    else:
                        slot = 4 * (r - 1) + (P % FOLDB) // 3
                        nc.scalar.copy(fb[:, slot, :, :], dot[:, :, 0:N2])
                        if P % FOLDB == FOLDB - 1:
                            pend += fold_batch(fb, bi)
                    if pend:
                        pend.pop(0)()
            for fn in pend:
                fn()

            nc.sync.dma_start(cmax_d[:, :], colmax[:, :, :])

    _split_multi_waits(nc)
    return nc


_NC_CACHE = None


def _get_nc():
    global _NC_CACHE
    if _NC_CACHE is None:
        _NC_CACHE = _build_nc()
    return _NC_CACHE


def _pack(x):
    """[B, S, C, N] fp32 -> [B, 128, NT*N2] bf16 (pad dup)."""
    x = np.concatenate([x, x[:, S - 1:S]], axis=1)          # pair 75 = dup 74
    x = np.concatenate([x, x[:, :, :, N - 1:N]], axis=3)    # col 361 = dup 360
    x = x.reshape(B, NT, 2, C, N2).transpose(0, 2, 3, 1, 4)  # [B,2,C,NT,N2]
    x = x.reshape(B, 128, NT * N2)
    return np.ascontiguousarray(x.astype(ml_dtypes.bfloat16))


def _prep(support_x, query_x):
    sx = np.asarray(support_x, dtype=np.float32).reshape(B, S, C, N)
    qx = np.asarray(query_x, dtype=np.float32).reshape(B, S, C, N)
    qn = np.sqrt(np.sum(qx * qx, axis=2))                   # [B,S,N]
    qhat = qx / np.maximum(qn, EPS)[:, :, None, :]
    sn = np.sqrt(np.sum(sx * sx, axis=2))                   # [B,S,N]
    rs = 1.0 / np.maximum(sn, EPS)
    return _pack(sx), _pack(qhat), rs


def _make_in_maps(support_x, query_x):
    sup, qh, rs = _prep(support_x, query_x)
    return [{"support": sup[b], "query": qh[b]} for b in range(B)], rs


def kernel(support_x, query_x, **_unused):
    in_maps, rs = _make_in_maps(support_x, query_x)
    nc = _get_nc()
    res = run_bass_kernel_spmd(nc, in_maps, core_ids=list(range(B)))

    i = np.arange(N)
    m, r = i // 128, i % 128
    out = np.empty((B, S, N), dtype=np.float32)
    for b in range(B):
        cm = np.asarray(res.results[b]["cmax"]).reshape(128, SP, 3)
        out[b] = cm[r, :, m].T[0:S] * rs[b]
    return np.ascontiguousarray(out)


# revision 6
# speedup vs baseline: 1.7143x; 1.0542x over previous
"""Trainium2 Bass kernel v4 for nn_Middle_Moudle_v3 (retrieval_knn).

Per episode (b, s): cosine similarity of every support spatial C-vector
against every query spatial C-vector, max over query positions.

  support_x, query_x: [8, 75, 64, 19, 19] fp32  ->  out [8, 75, 361] fp32

Data-parallel over batch (8 episodes -> 8 cores).

Design (v4):
  - Query is normalized ON HOST (fp32) and shipped as q-hat bf16; support
    norms are applied on host after the kernel (scale commutes with max).
    No on-device norm pipeline at all.
  - HAM fix: K=64 half-array matmuls never register as PE activity (v2
    trace: clock stuck at 1.2 GHz for the whole main phase).  The host
    ships the support TWICE with the opposite partition half zeroed
    (sup0: even pairs in partitions 0-63, zeros below; sup1: odd pairs in
    64-127, zeros above).  Every main matmul is then a genuine K=128
    full-array op -> HAM stays at 2.4 GHz (v3 trace: warm for the whole
    main phase, 153ns/MM).  Zeros shipped from host (+3.5MB DMA) so no
    memsets serialize ahead of the input DMA (v3: 18us startup stall).
  - Drain split across both PSUM-capable engines, interleaved so they
    run concurrently: P%4==0 pairs direct-reduce on DVE from PSUM; the
    other three of four are ACT-copied to SBUF bf16 and max-folded on
    DVE (overlap-fold chain or pool_max).
"""
import numpy as np
import ml_dtypes

import concourse.bass as bass
import concourse.mybir as mybir
import concourse.tile as tile
from concourse.bass_utils import run_bass_kernel_spmd

F32 = mybir.dt.float32
BF16 = mybir.dt.bfloat16
B = 8
S = 75
C = 64
N = 361
N2 = 362
SP = 76        # padded pairs
NT = 38        # two-pair tiles
NP = 75        # computed pairs (pad pair 75 skipped)
CHUNKS = [(0, 128), (128, 128), (256, 105)]
DGROUPS = [(0, 10), (10, 10), (20, 10), (30, 8)]
HEAT = 26      # K=128 heater matmuls to warm the PE clock during input DMA
EPS = 1e-8

SPANP = 12     # pairs per fold batch span; P%4==0 direct, 9 fold slots

_ws_ctr = [0]


def _split_multi_waits(nc):
    """Move all-but-one sync wait of each instruction onto injected
    InstEventSemaphore instructions (standalone sequencer waits)."""
    for f in nc.m.functions:
        for bb in f.blocks:
            insts = list(bb.instructions)
            out = []
            changed = False
            for ins in insts:
                si = ins.sync_info
                if si is not None and len(si.on_wait) > 1:
                    waits = list(si.on_wait)
                    for w in waits[:-1]:
                        _ws_ctr[0] += 1
                        ev = mybir.InstEventSemaphore(
                            name=f"wsplit_{_ws_ctr[0]}",
                            engine=ins.engine,
                            sync_info=mybir.SyncInfo(on_wait=[w], on_update=[]),
                        )
                        out.append(ev)
                    ins.sync_info = mybir.SyncInfo(
                        on_wait=[waits[-1]], on_update=list(si.on_update)
                    )
                    changed = True
                out.append(ins)
            if changed:
                bb.instructions = out


def _build_nc():
    win_np = np.zeros((128, 152), dtype=np.float32)
    win_np[:, 74] = 1.0

    nc = bass.Bass(target_bir_lowering=False)
    sup0_d = nc.dram_tensor("support0", [128, NT * N2], BF16,
                            kind="ExternalInput")
    sup1_d = nc.dram_tensor("support1", [128, NT * N2], BF16,
                            kind="ExternalInput")
    qry_d = nc.dram_tensor("query", [128, NT * N2], BF16, kind="ExternalInput")
    cmax_d = nc.dram_tensor("cmax", [128, 3 * SP], F32, kind="ExternalOutput")
    win_d = nc.inline_tensor(win_np, name="win")

    mx = mybir.AluOpType.max

    with tile.TileContext(nc) as tc:
        with tc.tile_pool(name="inp", bufs=1) as inp, \
             tc.tile_pool(name="work", bufs=1) as work, \
             tc.tile_pool(name="fbp", bufs=2) as fbp, \
             tc.tile_pool(name="psh", bufs=1, space="PSUM") as psh, \
             tc.tile_pool(name="psd", bufs=2, space="PSUM") as psd:

            # --- heater weights (K=128) land first; heat the PE clock
            # while the big input DMAs stream in.
            win32 = work.tile([128, 152], F32)
            nc.sync.dma_start(win32[:], win_d[:])
            win_sb = work.tile([128, 152], BF16)
            nc.vector.tensor_copy(win_sb[:], win32[:])

            sup0 = inp.tile([128, NT, N2], BF16)
            sup1 = inp.tile([128, NT, N2], BF16)
            qt = inp.tile([128, NT, N2], BF16)

            for (g0, T) in DGROUPS:
                a, b = g0 * N2, (g0 + T) * N2
                nc.sync.dma_start(sup0[:, g0:g0 + T, :], sup0_d[:, a:b])
                nc.sync.dma_start(sup1[:, g0:g0 + T, :], sup1_d[:, a:b])
                nc.gpsimd.dma_start(qt[:, g0:g0 + T, :], qry_d[:, a:b])

            colmax = work.tile([128, SP, 3], F32)
            dscr = work.tile([128, 3, N2], BF16)   # dcopy-probe scratch

            # --- heater: full-array K=128 matmuls into a scrap bank
            hbank = psh.tile([128, 512], F32)
            for h in range(HEAT):
                nc.tensor.matmul(hbank[:, 0:152], win_sb[:, 0:128],
                                 win_sb[:, 0:152], start=True, stop=True)

            pend = []

            def fold_batch_tt(fb, bi):
                """Overlap-fold chain + 3 strided tail reduces."""
                P0 = SPANP * bi
                steps = []
                for (hi, w) in ((180, 182), (90, 92), (46, 46), (22, 24)):
                    steps.append(lambda hi=hi, w=w: nc.vector.tensor_tensor(
                        out=fb[:, :, :, 0:w], in0=fb[:, :, :, 0:w],
                        in1=fb[:, :, :, hi:hi + w], op=mx))

                def tail(t):
                    cv = colmax[:, P0 + 1 + t, :]
                    out_ap = bass.AP(tensor=cv.tensor, offset=cv.offset,
                                     ap=[list(cv.ap[0]), [12, 3], [1, 3]])
                    nc.vector.tensor_reduce(
                        out_ap, fb[:, 3 * t:3 * t + 3, :, 0:24],
                        axis=mybir.AxisListType.X, op=mx)
                for t in range(3):
                    steps.append(lambda t=t: tail(t))
                return steps

            fb = None
            for j in range(NT):
                for e in range(2):
                    P = 2 * j + e
                    if P >= NP:
                        continue
                    supE = sup0 if e == 0 else sup1
                    dot = psd.tile([128, 3, 512], F32, tag="dot")
                    for m, (off, mc) in enumerate(CHUNKS):
                        nc.tensor.matmul(
                            dot[0:mc, m, 0:N2],
                            supE[:, j, off:off + mc],
                            qt[:, j, 0:N2],
                            start=True, stop=True,
                        )
                    r = P % 4
                    bi = P // SPANP
                    if P % SPANP == 0 and bi < 6:
                        fb = fbp.tile([128, 9, 3, N2], BF16, tag="fb",
                                      name=f"fb{bi}")
                    if P == 72:
                        # dcopy probe: DVE copy PSUM->SBUF bf16 + reduce
                        nc.vector.tensor_copy(dscr[:], dot[:, :, 0:N2])
                        nc.vector.tensor_reduce(
                            colmax[:, P, :], dscr[:],
                            axis=mybir.AxisListType.X, op=mx)
                    elif r == 0 or bi >= 6:
                        nc.vector.tensor_reduce(
                            colmax[:, P, :], dot[:, :, 0:N2],
                            axis=mybir.AxisListType.X, op=mx)
                    else:
                        slot = 3 * (r - 1) + (P % SPANP) // 4
                        nc.scalar.copy(fb[:, slot, :, :], dot[:, :, 0:N2])
                        if P % SPANP == SPANP - 1:
                            pend.extend(fold_batch_tt(fb, bi))
                    if P == 48:
                        # all colmax[:, 0:36] writers are issued by now
                        # (batch bi=2's deferred folds pop at P=36..42)
                        nc.sync.dma_start(cmax_d[:, 0:108],
                                          colmax[:, 0:36, :])
                    if pend:
                        pend.pop(0)()
            for fn in pend:
                fn()

            nc.sync.dma_start(cmax_d[:, 108:3 * SP], colmax[:, 36:SP, :])

    _split_multi_waits(nc)
    return nc


_NC_CACHE = None


def _get_nc():
    global _NC_CACHE
    if _NC_CACHE is None:
        _NC_CACHE = _build_nc()
    return _NC_CACHE


def _pack(x):
    """[B, S, C, N] fp32 -> [B, 128, NT*N2] bf16 (pad dup)."""
    x = np.concatenate([x, x[:, S - 1:S]], axis=1)          # pair 75 = dup 74
    x = np.concatenate([x, x[:, :, :, N - 1:N]], axis=3)    # col 361 = dup 360
    x = x.reshape(B, NT, 2, C, N2).transpose(0, 2, 3, 1, 4)  # [B,2,C,NT,N2]
    x = x.reshape(B, 128, NT * N2)
    return np.ascontiguousarray(x.astype(ml_dtypes.bfloat16))


def _prep(support_x, query_x):
    sx = np.asarray(support_x, dtype=np.float32).reshape(B, S, C, N)
    qx = np.asarray(query_x, dtype=np.float32).reshape(B, S, C, N)
    qn = np.sqrt(np.sum(qx * qx, axis=2))                   # [B,S,N]
    qhat = qx / np.maximum(qn, EPS)[:, :, None, :]
    sn = np.sqrt(np.sum(sx * sx, axis=2))                   # [B,S,N]
    rs = 1.0 / np.maximum(sn, EPS)
    sup = _pack(sx)
    sup0 = sup.copy()
    sup0[:, 64:128, :] = 0
    sup1 = sup
    sup1[:, 0:64, :] = 0
    return sup0, sup1, _pack(qhat), rs


def _make_in_maps(support_x, query_x):
    sup0, sup1, qh, rs = _prep(support_x, query_x)
    return [{"support0": sup0[b], "support1": sup1[b], "query": qh[b]}
            for b in range(B)], rs


def kernel(support_x, query_x, **_unused):
    in_maps, rs = _make_in_maps(support_x, query_x)
    nc = _get_nc()
    res = run_bass_kernel_spmd(nc, in_maps, core_ids=list(range(B)))

    i = np.arange(N)
    m, r = i // 128, i % 128
    out = np.empty((B, S, N), dtype=np.float32)
    for b in range(B):
        cm = np.asarray(res.results[b]["cmax"]).reshape(128, SP, 3)
        out[b] = cm[r, :, m].T[0:S] * rs[b]
    return np.ascontiguousarray(out)


# revision 8
# speedup vs baseline: 1.7679x; 1.0313x over previous
"""Trainium2 Bass kernel v4 for nn_Middle_Moudle_v3 (retrieval_knn).

Per episode (b, s): cosine similarity of every support spatial C-vector
against every query spatial C-vector, max over query positions.

  support_x, query_x: [8, 75, 64, 19, 19] fp32  ->  out [8, 75, 361] fp32

Data-parallel over batch (8 episodes -> 8 cores).

Design (v4):
  - Query is normalized ON HOST (fp32) and shipped as q-hat bf16; support
    norms are applied on host after the kernel (scale commutes with max).
    No on-device norm pipeline at all.
  - HAM fix: K=64 half-array matmuls never register as PE activity (v2
    trace: clock stuck at 1.2 GHz for the whole main phase).  The host
    ships the support TWICE with the opposite partition half zeroed
    (sup0: even pairs in partitions 0-63, zeros below; sup1: odd pairs in
    64-127, zeros above).  Every main matmul is then a genuine K=128
    full-array op -> HAM stays at 2.4 GHz (v3 trace: warm for the whole
    main phase, 153ns/MM).  Zeros shipped from host (+3.5MB DMA) so no
    memsets serialize ahead of the input DMA (v3: 18us startup stall).
  - Drain split across both PSUM-capable engines, interleaved so they
    run concurrently: P%4==0 pairs direct-reduce on DVE from PSUM; the
    other three of four are ACT-copied to SBUF bf16 and max-folded on
    DVE (overlap-fold chain or pool_max).
"""
import numpy as np
import ml_dtypes

import concourse.bass as bass
import concourse.mybir as mybir
import concourse.tile as tile
from concourse.bass_utils import run_bass_kernel_spmd

F32 = mybir.dt.float32
BF16 = mybir.dt.bfloat16
B = 8
S = 75
C = 64
N = 361
N2 = 362
SP = 76        # padded pairs
NT = 38        # two-pair tiles
NP = 75        # computed pairs (pad pair 75 skipped)
CHUNKS = [(0, 128), (128, 128), (256, 105)]
DGROUPS = [(0, 6), (6, 8), (14, 8), (22, 8), (30, 8)]
HEAT = 26      # K=128 heater matmuls to warm the PE clock during input DMA
EPS = 1e-8

SPANP = 12     # pairs per fold batch span; P%4==0 direct, 9 fold slots

_ws_ctr = [0]


def _split_multi_waits(nc):
    """Move all-but-one sync wait of each instruction onto injected
    InstEventSemaphore instructions (standalone sequencer waits)."""
    for f in nc.m.functions:
        for bb in f.blocks:
            insts = list(bb.instructions)
            out = []
            changed = False
            for ins in insts:
                si = ins.sync_info
                if si is not None and len(si.on_wait) > 1:
                    waits = list(si.on_wait)
                    for w in waits[:-1]:
                        _ws_ctr[0] += 1
                        ev = mybir.InstEventSemaphore(
                            name=f"wsplit_{_ws_ctr[0]}",
                            engine=ins.engine,
                            sync_info=mybir.SyncInfo(on_wait=[w], on_update=[]),
                        )
                        out.append(ev)
                    ins.sync_info = mybir.SyncInfo(
                        on_wait=[waits[-1]], on_update=list(si.on_update)
                    )
                    changed = True
                out.append(ins)
            if changed:
                bb.instructions = out


def _build_nc():
    win_np = np.zeros((128, 152), dtype=np.float32)
    win_np[:, 74] = 1.0

    nc = bass.Bass(target_bir_lowering=False)
    sup0_d = nc.dram_tensor("support0", [128, NT * N2], BF16,
                            kind="ExternalInput")
    sup1_d = nc.dram_tensor("support1", [128, NT * N2], BF16,
                            kind="ExternalInput")
    qry_d = nc.dram_tensor("query", [128, NT * N2], BF16, kind="ExternalInput")
    cmax_d = nc.dram_tensor("cmax", [128, 3 * SP], F32, kind="ExternalOutput")
    win_d = nc.inline_tensor(win_np, name="win")

    mx = mybir.AluOpType.max

    with tile.TileContext(nc) as tc:
        with tc.tile_pool(name="inp", bufs=1) as inp, \
             tc.tile_pool(name="work", bufs=1) as work, \
             tc.tile_pool(name="fbp", bufs=2) as fbp, \
             tc.tile_pool(name="psh", bufs=1, space="PSUM") as psh, \
             tc.tile_pool(name="psd", bufs=2, space="PSUM") as psd:

            # --- heater weights (K=128) land first; heat the PE clock
            # while the big input DMAs stream in.
            win32 = work.tile([128, 152], F32)
            nc.sync.dma_start(win32[:], win_d[:])
            win_sb = work.tile([128, 152], BF16)
            nc.vector.tensor_copy(win_sb[:], win32[:])

            sup0 = inp.tile([128, NT, N2], BF16)
            sup1 = inp.tile([128, NT, N2], BF16)
            qt = inp.tile([128, NT, N2], BF16)

            for (g0, T) in DGROUPS:
                a, b = g0 * N2, (g0 + T) * N2
                nc.sync.dma_start(sup0[:, g0:g0 + T, :], sup0_d[:, a:b])
                nc.scalar.dma_start(sup1[:, g0:g0 + T, :], sup1_d[:, a:b])
                nc.gpsimd.dma_start(qt[:, g0:g0 + T, :], qry_d[:, a:b])

            colmax = work.tile([128, SP, 3], F32)

            # --- heater: full-array K=128 matmuls into a scrap bank
            hbank = psh.tile([128, 512], F32)
            for h in range(HEAT):
                nc.tensor.matmul(hbank[:, 0:152], win_sb[:, 0:128],
                                 win_sb[:, 0:152], start=True, stop=True)

            pend = []

            def fold_batch_tt(fb, bi):
                """Overlap-fold chain + 3 strided tail reduces.  On odd
                spans the last two slots fold on GPSIMD (idle engine)."""
                P0 = SPANP * bi
                steps = []
                for (hi, w) in ((180, 182), (90, 92), (46, 46), (22, 24)):
                    steps.append(lambda hi=hi, w=w: nc.vector.tensor_tensor(
                        out=fb[:, :, :, 0:w], in0=fb[:, :, :, 0:w],
                        in1=fb[:, :, :, hi:hi + w], op=mx))

                def tail(t):
                    cv = colmax[:, P0 + 1 + t, :]
                    out_ap = bass.AP(tensor=cv.tensor, offset=cv.offset,
                                     ap=[list(cv.ap[0]), [12, 3], [1, 3]])
                    nc.vector.tensor_reduce(
                        out_ap, fb[:, 3 * t:3 * t + 3, :, 0:24],
                        axis=mybir.AxisListType.X, op=mx)
                for t in range(3):
                    steps.append(lambda t=t: tail(t))
                return steps

            fb = None
            for j in range(NT):
                for e in range(2):
                    P = 2 * j + e
                    if P >= NP:
                        continue
                    supE = sup0 if e == 0 else sup1
                    dot = psd.tile([128, 3, 512], F32, tag="dot")
                    for m, (off, mc) in enumerate(CHUNKS):
                        nc.tensor.matmul(
                            dot[0:mc, m, 0:N2],
                            supE[:, j, off:off + mc],
                            qt[:, j, 0:N2],
                            start=True, stop=True,
                        )
                    r = P % 4
                    bi = P // SPANP
                    if P % SPANP == 0 and bi < 6:
                        fb = fbp.tile([128, 9, 3, N2], BF16, tag="fb",
                                      name=f"fb{bi}")
                    if r == 0 or bi >= 6:
                        nc.vector.tensor_reduce(
                            colmax[:, P, :], dot[:, :, 0:N2],
                            axis=mybir.AxisListType.X, op=mx)
                    else:
                        slot = 3 * (r - 1) + (P % SPANP) // 4
                        nc.scalar.copy(fb[:, slot, :, :], dot[:, :, 0:N2])
                        if P % SPANP == SPANP - 1:
                            pend.extend(fold_batch_tt(fb, bi))
                    if P == 48:
                        # all colmax[:, 0:36] writers are issued by now
                        # (batch bi=2's deferred folds pop at P=36..42)
                        nc.sync.dma_start(cmax_d[:, 0:108],
                                          colmax[:, 0:36, :])
                    if P == 72:
                        nc.sync.dma_start(cmax_d[:, 108:180],
                                          colmax[:, 36:60, :])
                    if pend:
                        pend.pop(0)()
                        if P >= 56 and pend:
                            pend.pop(0)()
            for fn in pend:
                fn()

            nc.sync.dma_start(cmax_d[:, 180:3 * SP], colmax[:, 60:SP, :])

    _split_multi_waits(nc)
    return nc


_NC_CACHE = None


def _get_nc():
    global _NC_CACHE
    if _NC_CACHE is None:
        _NC_CACHE = _build_nc()
    return _NC_CACHE


def _pack(x):
    """[B, S, C, N] fp32 -> [B, 128, NT*N2] bf16 (pad dup)."""
    x = np.concatenate([x, x[:, S - 1:S]], axis=1)          # pair 75 = dup 74
    x = np.concatenate([x, x[:, :, :, N - 1:N]], axis=3)    # col 361 = dup 360
    x = x.reshape(B, NT, 2, C, N2).transpose(0, 2, 3, 1, 4)  # [B,2,C,NT,N2]
    x = x.reshape(B, 128, NT * N2)
    return np.ascontiguousarray(x.astype(ml_dtypes.bfloat16))


def _prep(support_x, query_x):
    sx = np.asarray(support_x, dtype=np.float32).reshape(B, S, C, N)
    qx = np.asarray(query_x, dtype=np.float32).reshape(B, S, C, N)
    qn = np.sqrt(np.sum(qx * qx, axis=2))                   # [B,S,N]
    qhat = qx / np.maximum(qn, EPS)[:, :, None, :]
    sn = np.sqrt(np.sum(sx * sx, axis=2))                   # [B,S,N]
    rs = 1.0 / np.maximum(sn, EPS)
    sup = _pack(sx)
    sup0 = sup.copy()
    sup0[:, 64:128, :] = 0
    sup1 = sup
    sup1[:, 0:64, :] = 0
    return sup0, sup1, _pack(qhat), rs


def _make_in_maps(support_x, query_x):
    sup0, sup1, qh, rs = _prep(support_x, query_x)
    return [{"support0": sup0[b], "support1": sup1[b], "query": qh[b]}
            for b in range(B)], rs


def kernel(support_x, query_x, **_unused):
    in_maps, rs = _make_in_maps(support_x, query_x)
    nc = _get_nc()
    res = run_bass_kernel_spmd(nc, in_maps, core_ids=list(range(B)))

    i = np.arange(N)
    m, r = i // 128, i % 128
    out = np.empty((B, S, N), dtype=np.float32)
    for b in range(B):
        cm = np.asarray(res.results[b]["cmax"]).reshape(128, SP, 3)
        out[b] = cm[r, :, m].T[0:S] * rs[b]
    return np.ascontiguousarray(out)


# revision 11
# speedup vs baseline: 1.7949x; 1.0153x over previous
"""Trainium2 Bass kernel v7 for nn_Middle_Moudle_v3 (retrieval_knn).

Per episode (b, s): cosine similarity of every support spatial C-vector
against every query spatial C-vector, max over query positions.

  support_x, query_x: [8, 75, 64, 19, 19] fp32  ->  out [8, 75, 361] fp32

Data-parallel over batch (8 episodes -> 8 cores).

Design (v7):
  - Query is normalized ON HOST (fp32) and shipped as q-hat bf16; support
    norms are applied on host after the kernel (scale commutes with max).
    No on-device norm pipeline at all.
  - HAM fix: K=64 half-array matmuls never register as PE activity (v2
    trace: clock stuck at 1.2 GHz the whole main phase).  The host ships
    the support TWICE with the opposite partition half zeroed (sup0:
    even pairs in partitions 0-63, zeros below; sup1: odd pairs in
    64-127, zeros above).  Every main matmul is then a genuine K=128
    full-array op -> HAM stays at 2.4 GHz (153ns/MM).
  - Drain split tuned from trace so both PSUM-capable engines run
    concurrently at equal load: per 12-pair span, P%4==0 direct-reduces
    on DVE from PSUM (1490ns); the other 9 are ACT-copied to SBUF bf16
    (1163ns) and max-folded on DVE (overlap-fold ladder, ~650ns/pair).
  - Input DMA spread over three HW queues (sync/tensor/vector issue) so
    each carries ~3.5MB and the first tile group lands early; ACT's
    queue carries no DMA triggers.
"""
import numpy as np
import ml_dtypes

import concourse.bass as bass
import concourse.mybir as mybir
import concourse.tile as tile
from concourse.bass_utils import run_bass_kernel_spmd

F32 = mybir.dt.float32
BF16 = mybir.dt.bfloat16
B = 8
S = 75
C = 64
N = 361
N2 = 362
SP = 76        # padded pairs
NT = 38        # two-pair tiles
NP = 75        # computed pairs (pad pair 75 skipped)
CHUNKS = [(0, 128), (128, 128), (256, 105)]
DGROUPS = [(0, 3), (3, 5), (8, 6), (14, 6), (20, 6), (26, 6), (32, 6)]
HEAT = 26      # K=128 heater matmuls to warm the PE clock during input DMA
EPS = 1e-8

SPANP = 12     # pairs per span: P%4==0 direct, 9 fold slots by residue
NSPAN = 6

_ws_ctr = [0]


def _split_multi_waits(nc):
    """Move all-but-one sync wait of each instruction onto injected
    InstEventSemaphore instructions (standalone sequencer waits)."""
    for f in nc.m.functions:
        for bb in f.blocks:
            insts = list(bb.instructions)
            out = []
            changed = False
            for ins in insts:
                si = ins.sync_info
                if si is not None and len(si.on_wait) > 1:
                    waits = list(si.on_wait)
                    for w in waits[:-1]:
                        _ws_ctr[0] += 1
                        ev = mybir.InstEventSemaphore(
                            name=f"wsplit_{_ws_ctr[0]}",
                            engine=ins.engine,
                            sync_info=mybir.SyncInfo(on_wait=[w], on_update=[]),
                        )
                        out.append(ev)
                    ins.sync_info = mybir.SyncInfo(
                        on_wait=[waits[-1]], on_update=list(si.on_update)
                    )
                    changed = True
                out.append(ins)
            if changed:
                bb.instructions = out


def _build_nc():
    win_np = np.zeros((128, 152), dtype=np.float32)
    win_np[:, 74] = 1.0

    nc = bass.Bass(target_bir_lowering=False)
    sup0_d = nc.dram_tensor("support0", [128, NT * N2], BF16,
                            kind="ExternalInput")
    sup1_d = nc.dram_tensor("support1", [128, NT * N2], BF16,
                            kind="ExternalInput")
    qry_d = nc.dram_tensor("query", [128, NT * N2], BF16, kind="ExternalInput")
    cmax_d = nc.dram_tensor("cmax", [128, 3 * SP], F32, kind="ExternalOutput")
    win_d = nc.inline_tensor(win_np, name="win")

    mx = mybir.AluOpType.max

    with tile.TileContext(nc) as tc:
        with tc.tile_pool(name="inp", bufs=1) as inp, \
             tc.tile_pool(name="work", bufs=1) as work, \
             tc.tile_pool(name="fbp", bufs=2) as fbp, \
             tc.tile_pool(name="psh", bufs=1, space="PSUM") as psh, \
             tc.tile_pool(name="psd", bufs=2, space="PSUM") as psd:

            # --- heater weights (K=128) land first; heat the PE clock
            # while the big input DMAs stream in.
            win32 = work.tile([128, 152], F32)
            nc.sync.dma_start(win32[:], win_d[:])
            win_sb = work.tile([128, 152], BF16)
            nc.vector.tensor_copy(win_sb[:], win32[:])

            sup0 = inp.tile([128, NT, N2], BF16)
            sup1 = inp.tile([128, NT, N2], BF16)
            qt = inp.tile([128, NT, N2], BF16)

            for (g0, T) in DGROUPS:
                a, b = g0 * N2, (g0 + T) * N2
                nc.sync.dma_start(sup0[:, g0:g0 + T, :], sup0_d[:, a:b])
                nc.scalar.dma_start(sup1[:, g0:g0 + T, :], sup1_d[:, a:b])
                nc.gpsimd.dma_start(qt[:, g0:g0 + T, :], qry_d[:, a:b])

            colmax = work.tile([128, SP, 3], F32)

            # --- heater: full-array K=128 matmuls into a scrap bank
            hbank = psh.tile([128, 512], F32)
            for h in range(HEAT):
                nc.tensor.matmul(hbank[:, 0:152], win_sb[:, 0:128],
                                 win_sb[:, 0:152], start=True, stop=True)

            pend = []

            def fold_chain(fb, s0, ns):
                """Overlap-fold ladder on fb slots [s0, s0+ns)."""
                steps = []
                for (hi, w) in ((180, 182), (90, 92), (46, 46), (22, 24)):
                    steps.append(lambda hi=hi, w=w: nc.vector.tensor_tensor(
                        out=fb[:, s0:s0 + ns, :, 0:w],
                        in0=fb[:, s0:s0 + ns, :, 0:w],
                        in1=fb[:, s0:s0 + ns, :, hi:hi + w], op=mx))
                return steps

            def tails(fb, bi, tlist):
                """Strided tail reduces: slots 3t..3t+2 are pairs
                P0+{4t+1..4t+3} ... wait, slots by residue r=1..3:
                slot 3(r-1)+k covers pair P0 + 4k + r."""
                P0 = SPANP * bi
                steps = []

                def tail(t, nf):
                    cv = colmax[:, P0 + 1 + t, :]
                    out_ap = bass.AP(tensor=cv.tensor, offset=cv.offset,
                                     ap=[list(cv.ap[0]), [12, nf], [1, 3]])
                    nc.vector.tensor_reduce(
                        out_ap, fb[:, 3 * t:3 * t + nf, :, 0:24],
                        axis=mybir.AxisListType.X, op=mx)
                for (t, nf) in tlist:
                    steps.append(lambda t=t, nf=nf: tail(t, nf))
                return steps

            fb = None
            for j in range(NT):
                for e in range(2):
                    P = 2 * j + e
                    if P >= NP:
                        continue
                    supE = sup0 if e == 0 else sup1
                    dot = psd.tile([128, 3, 512], F32, tag="dot")
                    for m, (off, mc) in enumerate(CHUNKS):
                        nc.tensor.matmul(
                            dot[0:mc, m, 0:N2],
                            supE[:, j, off:off + mc],
                            qt[:, j, 0:N2],
                            start=True, stop=True,
                        )
                    r = P % 4
                    pj = P % SPANP
                    bi = P // SPANP
                    if pj == 0 and bi < NSPAN:
                        fb = fbp.tile([128, 9, 3, N2], BF16, tag="fb",
                                      name=f"fb{bi}")
                    if r == 0 or bi >= NSPAN:
                        nc.vector.tensor_reduce(
                            colmax[:, P, :], dot[:, :, 0:N2],
                            axis=mybir.AxisListType.X, op=mx)
                    else:
                        # slot by residue: 3*(r-1) + pj//4
                        nc.scalar.copy(fb[:, 3 * (r - 1) + pj // 4, :, :],
                                       dot[:, :, 0:N2])
                        if pj == SPANP - 1:
                            pend.extend(fold_chain(fb, 0, 9))
                            pend.extend(tails(fb, bi,
                                              [(0, 3), (1, 3), (2, 3)]))
                    if P == 48:
                        # all colmax[:, 0:36] writers are issued by now
                        nc.sync.dma_start(cmax_d[:, 0:108],
                                          colmax[:, 0:36, :])
                    if P == 72:
                        nc.sync.dma_start(cmax_d[:, 108:180],
                                          colmax[:, 36:60, :])
                    if pend:
                        pend.pop(0)()
                        if P >= 44 and pend:
                            pend.pop(0)()
            for fn in pend:
                fn()

            nc.sync.dma_start(cmax_d[:, 180:3 * SP], colmax[:, 60:SP, :])

    _split_multi_waits(nc)
    return nc


_NC_CACHE = None


def _get_nc():
    global _NC_CACHE
    if _NC_CACHE is None:
        _NC_CACHE = _build_nc()
    return _NC_CACHE


def _pack(x):
    """[B, S, C, N] fp32 -> [B, 128, NT*N2] bf16 (pad dup)."""
    x = np.concatenate([x, x[:, S - 1:S]], axis=1)          # pair 75 = dup 74
    x = np.concatenate([x, x[:, :, :, N - 1:N]], axis=3)    # col 361 = dup 360
    x = x.reshape(B, NT, 2, C, N2).transpose(0, 2, 3, 1, 4)  # [B,2,C,NT,N2]
    x = x.reshape(B, 128, NT * N2)
    return np.ascontiguousarray(x.astype(ml_dtypes.bfloat16))


def _prep(support_x, query_x):
    sx = np.asarray(support_x, dtype=np.float32).reshape(B, S, C, N)
    qx = np.asarray(query_x, dtype=np.float32).reshape(B, S, C, N)
    qn = np.sqrt(np.sum(qx * qx, axis=2))                   # [B,S,N]
    qhat = qx / np.maximum(qn, EPS)[:, :, None, :]
    sn = np.sqrt(np.sum(sx * sx, axis=2))                   # [B,S,N]
    rs = 1.0 / np.maximum(sn, EPS)
    sup = _pack(sx)
    sup0 = sup.copy()
    sup0[:, 64:128, :] = 0
    sup1 = sup
    sup1[:, 0:64, :] = 0
    return sup0, sup1, _pack(qhat), rs


def _make_in_maps(support_x, query_x):
    sup0, sup1, qh, rs = _prep(support_x, query_x)
    return [{"support0": sup0[b], "support1": sup1[b], "query": qh[b]}
            for b in range(B)], rs


def kernel(support_x, query_x, **_unused):
    in_maps, rs = _make_in_maps(support_x, query_x)
    nc = _get_nc()
    res = run_bass_kernel_spmd(nc, in_maps, core_ids=list(range(B)))

    i = np.arange(N)
    m, r = i // 128, i % 128
    out = np.empty((B, S, N), dtype=np.float32)
    for b in range(B):
        cm = np.asarray(res.results[b]["cmax"]).reshape(128, SP, 3)
        out[b] = cm[r, :, m].T[0:S] * rs[b]
    return np.ascontiguousarray(out)
